# revision 13
# baseline (speedup 1.0000x reference)
"""Trainium2 Bass kernel for ContextualAttention (sparse_attention).

Problem (hardcoded shapes): f [B=2, C=128, H=128, W=128] fp32.
  f_s = f[:, :, ::2, ::2]  (64x64, L=4096 patches)
  w   = 3x3 patches of f_s (the matching filters), wn = w/||w||
  scores[l,p] = <wn_l, x_p>  (x = 3x3 patches of f_s)  -> [L, L] Gram-like
  att = softmax(10*scores, axis=l)
  y   = conv_transpose2d(att, raw 4x4 patches of f, stride 2, pad 1) / 4

Sharding: 8 cores = 2 batches x 4 query-blocks (1024 queries each).
Each core computes scores[l, p_block] directly in [l-on-partitions, p] layout
(matmul operands are contiguous AP views of SBUF-resident fp16 shift-planes),
applies a Cauchy-Schwarz-stable softmax (exp(s*10/||w_l|| - 10*||x_p||) <= e^0,
provably no overflow; softmax over l is invariant to the per-column shift),
then runs the deconv GEMM P_ij[c,p] = sum_l R_ij[l,c] * E[l,p] with R tiles
produced by contiguous xbar DMA transposes of row/column-parity planes of f.
The scaled planes are scatter-added into a per-core output slab; the host
overlap-adds the slabs.
"""

import numpy as np

import concourse.bacc as bacc
import concourse.bass as bass
import concourse.mybir as mybir
import concourse.tile as tile
from concourse.bass_utils import run_bass_kernel_spmd

F32 = mybir.dt.float32
F16 = mybir.dt.float16
AF = mybir.ActivationFunctionType
OP = mybir.AluOpType

B, C, H, W = 2, 128, 128, 128
Hs = Ws = 64
L = Hs * Ws                    # 4096
QBLK = 4                       # query blocks per batch
QROWS = Hs // QBLK             # 16 h-rows of queries per core
PPC = QROWS * Ws               # 1024 queries per core
HSP, WSP = Hs + 2, Ws + 2      # 66 (low-res, pad 1 all sides)
FQ = QROWS + 2                 # 18 query rows incl. halo
SLAB_R, SLAB_C = 2 * QROWS + 2, 2 * Ws + 2   # 34 x 130 output slab
NLT = L // 128                 # 32 l-tiles of 128
NPC = PPC // 512               # 2 p-chunks of 512


def _norm_chunk(nc, pool, psum_pool, ones_t, plane, row0, nrows_used):
    """Partition-sum of 3x3-shifted squares -> PSUM [1, nrows_used*64].

    plane: [128, 3, R, 64] shift-planes (index j), rows row0..row0+nrows_used+2
    give the 3x3 patch sums for `nrows_used` h-rows of patches."""
    n = nrows_used * Ws
    sqc = pool.tile([128, 3, nrows_used + 2, Ws], F16, name="sqc", tag="sqc")
    for j in range(3):
        nc.scalar.square(sqc[:, j], plane[:, j, row0: row0 + nrows_used + 2, :])
    ps = psum_pool.tile([1, n], F32, name="ps_nrm", tag="ps")
    for idx in range(9):
        i, j = idx // 3, idx % 3
        nc.tensor.matmul(
            ps, ones_t, sqc[:, j, i: i + nrows_used, :],
            start=(idx == 0), stop=(idx == 8),
        )
    return ps


def _build_body(nc, tc, ctx, fb, fq, out_e, r10_d, b_d, rz_d):
    main = ctx.enter_context(tc.tile_pool(name="main", bufs=1))
    kpl = main.tile([128, 2, 4, 65, 64], F16, name="kpl")    # parity planes [c,a,j,u,w]
    r10_l = main.tile([128, NLT], F32, name="r10_l")         # 10/||w_l|| per-partition
    rz_b = main.tile([128, PPC], F32, name="rz_b")           # 0.25/Z bcast rows
    ones_t = main.tile([128, 1], F16, name="ones_t")
    eep = ctx.enter_context(tc.tile_pool(name="eep", bufs=1))
    ee = eep.tile([128, NLT, PPC], F16, name="ee")           # E (unnormalized att)

    nc.vector.memset(ones_t, 1.0)

    # ---------------- phase 0: load f, build full-res parity planes ----------------
    with tc.tile_pool(name="prep", bufs=1) as prep:
        f16c = prep.tile([128, H, W], F16, name="f16c")
        nc.gpsimd.dma_start(out=f16c[:, :, :], in_=fb[:, :, :])  # f32->f16 cast

        # kpl[c,a,j,u,w] = f_pad1[c, 2u+a, 2w+j] = f[c, 2u+a-1, 2w+j-1]
        nc.vector.memset(kpl[:, 0, :, 0, :], 0.0)    # a=0, u=0  -> src row -1
        nc.vector.memset(kpl[:, 1, :, 64, :], 0.0)   # a=1, u=64 -> src row 128
        nc.vector.memset(kpl[:, :, 0, :, 0], 0.0)    # j=0, w=0  -> src col -1
        nc.vector.memset(kpl[:, :, 3, :, 63], 0.0)   # j=3, w=63 -> src col 128
        for a in range(2):
            u_lo, u_hi = (1, 65) if a == 0 else (0, 64)
            r_lo = 2 * u_lo + a - 1
            for j in range(4):
                w_lo, w_hi = (1 if j == 0 else 0), (63 if j == 3 else 64)
                c_lo = 2 * w_lo + j - 1
                nc.vector.tensor_copy(
                    kpl[:, a, j, u_lo:u_hi, w_lo:w_hi],
                    f16c[:, r_lo: r_lo + 2 * (u_hi - u_lo) - 1: 2,
                         c_lo: c_lo + 2 * (w_hi - w_lo) - 1: 2],
                )

    # ------- phases 1-2: low-res shift planes, norms, scores, Z -------
    with tc.tile_pool(name="planes", bufs=1) as planes:
        # Lj[c,j,y,w] = fsp[c, y, w+j] where fsp = pad1(f[::2,::2]) [66x66]
        # interior from kpl[a=1,j=1]: fsp[y,x] = kpl[c,1,1,y-1,x-1], x-1 in [0,64)
        lj = planes.tile([128, 3, HSP, Ws], F16, name="lj")
        nc.vector.memset(lj, 0.0)
        for j in range(3):
            w_lo = 1 if j == 0 else 0
            w_hi = min(64, 65 - j)
            nc.vector.tensor_copy(
                lj[:, j, 1:65, w_lo:w_hi],
                kpl[:, 1, 1, 0:64, w_lo + j - 1: w_hi + j - 1],
            )
        # Lq[c,j,y,w] = fq[c, y, w+j]; fq is already padded on the host
        lq = planes.tile([128, 3, FQ, Ws], F16, name="lq")
        fq32 = planes.tile([128, FQ, WSP], F32, name="fq32")
        nc.sync.dma_start(out=fq32[:, :, :], in_=fq[:, :, :])
        for j in range(3):
            nc.vector.tensor_copy(lq[:, j], fq32[:, :, j: j + Ws])  # f32->f16
        b_b = planes.tile([128, PPC], F32, name="b_b")       # 10*||x_p|| bcast rows

        # -------- norms --------
        with (
            tc.tile_pool(name="npsum", bufs=2, space="PSUM") as npsum,
            tc.tile_pool(name="ntmp", bufs=3) as ntmp,
        ):
            for ch in range(8):   # ||w_l||, 512 l's per chunk
                ps = _norm_chunk(nc, ntmp, npsum, ones_t, lj, ch * 8, 8)
                tmp = ntmp.tile([1, 512], F32, name="tmp_n", tag="t")
                # sqrt(0.01*n2) = ||w||/10 ; reciprocal -> 10/||w||
                nc.scalar.activation(tmp, ps, AF.Sqrt, scale=0.01)
                tmp2 = ntmp.tile([1, 512], F32, name="tmp_n2", tag="t")
                nc.vector.reciprocal(tmp2, tmp)
                nc.sync.dma_start(out=r10_d[:, ch * 512:(ch + 1) * 512], in_=tmp2)
            for pc in range(NPC):  # 10*||x_p||
                ps = _norm_chunk(nc, ntmp, npsum, ones_t, lq, pc * 8, 8)
                tmp = ntmp.tile([1, 512], F32, name="tmp_b", tag="t")
                nc.scalar.activation(tmp, ps, AF.Sqrt, scale=100.0)
                nc.sync.dma_start(out=b_d[:, pc * 512:(pc + 1) * 512], in_=tmp)

        # load back in partition layouts: r10_l[p, t] = r10_row[t*128 + p]
        nc.sync.dma_start(out=r10_l, in_=r10_d[0, :].rearrange("(t p) -> p t", p=128))
        nc.sync.dma_start(out=b_b, in_=b_d[0:1, :].partition_broadcast(128)[:, 0, :])

        # -------- scores -> E --------
        with (
            tc.tile_pool(name="spsum", bufs=6, space="PSUM") as spsum,
            tc.tile_pool(name="stmp", bufs=4) as stmp,
        ):
            for lt in range(NLT):
                for pc in range(NPC):
                    ps = spsum.tile([128, 512], F32, name="ps_s")
                    for idx in range(9):
                        i, j = idx // 3, idx % 3
                        lhsT = lj[:, j, 2 * lt + i: 2 * lt + 2 + i, :]   # [c, 2, 64]
                        rhs = lq[:, j, 8 * pc + i: 8 * pc + 8 + i, :]    # [c, 8, 64]
                        nc.tensor.matmul(ps, lhsT, rhs, start=(idx == 0), stop=(idx == 8))
                    t1 = stmp.tile([128, 512], F32, name="t1")
                    nc.vector.scalar_tensor_tensor(
                        out=t1, in0=ps, scalar=r10_l[:, lt:lt + 1],
                        in1=b_b[:, pc * 512:(pc + 1) * 512],
                        op0=OP.mult, op1=OP.subtract,
                    )
                    nc.scalar.activation(ee[:, lt, pc * 512:(pc + 1) * 512], t1, AF.Exp)

        # -------- Z = sum_l E --------
        with (
            tc.tile_pool(name="zpsum", bufs=1, space="PSUM") as zpsum,
            tc.tile_pool(name="ztmp", bufs=1) as ztmp,
        ):
            rz_row = ztmp.tile([1, PPC], F32, name="rz_row")
            for pc in range(NPC):
                psz = zpsum.tile([1, 512], F32, name="ps_z", tag="psz")
                for lt in range(NLT):
                    nc.tensor.matmul(
                        psz, ones_t, ee[:, lt, pc * 512:(pc + 1) * 512],
                        start=(lt == 0), stop=(lt == NLT - 1),
                    )
                z4 = ztmp.tile([1, 512], F32, name="z4")
                nc.scalar.mul(z4, psz, 4.0)
                nc.vector.reciprocal(rz_row[:, pc * 512:(pc + 1) * 512], z4)
            nc.sync.dma_start(out=rz_d[:, :], in_=rz_row)
            nc.sync.dma_start(out=rz_b, in_=rz_d[0:1, :].partition_broadcast(128)[:, 0, :])

    # ---------------- phase 3: deconv + scatter-add ----------------
    slab_pool = ctx.enter_context(tc.tile_pool(name="slabp", bufs=1))
    slab = slab_pool.tile([128, SLAB_R, SLAB_C], F32, name="slab")
    # DVE memset so every slab writer is DVE -> single wait on the final store
    nc.vector.memset(slab, 0.0)

    with (
        tc.tile_pool(name="rtp", bufs=2) as rtp,
        tc.tile_pool(name="dpsum", bufs=8, space="PSUM") as dpsum,
        tc.tile_pool(name="dtmp", bufs=4) as dtmp,
    ):
        for i in range(4):
            a, di = i & 1, i >> 1
            for j in range(4):
                rt = rtp.tile([128, NLT, 128], F16, name="rt", tag="rt")
                for lc in range(NLT):
                    u0 = 2 * lc + di
                    # src [c, u0:u0+2, :] merges to contiguous [c, 128]
                    nc.sync.dma_start_transpose(rt[:, lc, :], kpl[:, a, j, u0:u0 + 2, :])
                for pc in range(NPC):
                    ps = dpsum.tile([128, 512], F32, name="ps_d")
                    for lc in range(NLT):
                        nc.tensor.matmul(
                            ps, rt[:, lc, :], ee[:, lc, pc * 512:(pc + 1) * 512],
                            start=(lc == 0), stop=(lc == NLT - 1),
                        )
                    tmp = dtmp.tile([128, 8, Ws], F32, name="tmp_d")
                    nc.vector.tensor_mul(
                        tmp, ps.rearrange("c (h w) -> c h w", h=8),
                        rz_b[:, pc * 512:(pc + 1) * 512].rearrange("c (h w) -> c h w", h=8),
                    )
                    view = slab[:, 16 * pc + i: 16 * pc + i + 15: 2, j: j + 127: 2]
                    nc.vector.tensor_add(view, view, tmp)

    nc.sync.dma_start(out=out_e[:, :, :], in_=slab)


def build_nc(reps=1):
    """reps>1 repeats the whole body (serialized via WAW on the DRAM
    tensors) -- used only to wall-clock the marginal per-rep HW time."""
    from contextlib import ExitStack

    nc = bacc.Bacc(None)
    fb = nc.dram_tensor("fb", [C, H, W], F32, kind="ExternalInput")
    fq = nc.dram_tensor("fq", [C, FQ, WSP], F32, kind="ExternalInput")
    out_e = nc.dram_tensor("out", [C, SLAB_R, SLAB_C], F32, kind="ExternalOutput")
    r10_d = nc.dram_tensor("r10_d", [1, L], F32)
    b_d = nc.dram_tensor("b_d", [1, PPC], F32)
    rz_d = nc.dram_tensor("rz_d", [1, PPC], F32)

    with ExitStack() as ctx:
        tc = ctx.enter_context(tile.TileContext(nc))
        for _ in range(reps):
            with ExitStack() as rep_ctx:
                _build_body(nc, tc, rep_ctx, fb, fq, out_e, r10_d, b_d, rz_d)
    nc.compile()   # bacc: splits sync waits to <=1 per instruction (TRN2 limit)
    return nc


_NC_CACHE = None


def kernel(f: np.ndarray) -> np.ndarray:
    global _NC_CACHE
    f = np.ascontiguousarray(np.asarray(f, dtype=np.float32))
    assert f.shape == (B, C, H, W), f.shape

    if _NC_CACHE is None:
        _NC_CACHE = build_nc()
    nc = _NC_CACHE

    in_maps = []
    for core in range(8):
        b, q = core // 4, core % 4
        fs_pad = np.zeros((C, HSP, WSP), np.float32)
        fs_pad[:, 1:Hs + 1, 1:Ws + 1] = f[b][:, ::2, ::2]
        fq_arr = np.ascontiguousarray(fs_pad[:, q * QROWS: q * QROWS + FQ, :])
        in_maps.append({"fb": np.ascontiguousarray(f[b]), "fq": fq_arr})

    res = run_bass_kernel_spmd(nc, in_maps, core_ids=list(range(8)))
    results = res.results

    canvas = np.zeros((B, C, H + 4, W + 4), np.float32)
    for core in range(8):
        b, q = core // 4, core % 4
        slab = results[core]["out"]
        y0 = 2 * (q * QROWS) - 1 + 2       # slab row 0 in canvas coords (canvas pad 2)
        canvas[b, :, y0:y0 + SLAB_R, 1:1 + SLAB_C] += slab
    return np.ascontiguousarray(canvas[:, :, 2:2 + H, 2:2 + W])
